# revision 2
# baseline (speedup 1.0000x reference)
"""Trainium2 Bass kernel for nn_FFTMemAutoEncoderBranch (retrieval_knn).

Data-parallel over batch: 8 cores x 16 images, no cross-core communication.

Numerics: the problem's top-5 retrieval runs on near-identical queries (white
-noise FFT magnitudes), with 5th/6th similarity gaps down to 7e-6 — so conv
weights and DFT matrices must act at ~fp32 fidelity while activations tolerate
bf16. Scheme (validated against the reference in host emulation, 0/128 top-5
flips, ~2.7e-6 output rel err):
  - activations/staging in bf16
  - every stationary operand (DFT matrices G, conv weights) is split
    W = hi + lo into two bf16 matmuls accumulating in fp32 PSUM
  - retrieval + decoder in fp32

Structure per image:
  - FFT2 as DFT matmuls: z = G x G^T, G = roll(F,128,0)/16 (fftshift + ortho
    folded in; the batch component of jnp.fft.fftshift is a host batch roll).
  - conv1: strips of 4 output rows, M=(4t x 32ch), K=6 rows, 3 dx-offset
    matmuls; rhs staged via a DRAM bounce buffer (strided row gather).
  - conv2: M=(2t x 64ch), K=(32ci x 4j)=128; rhs via 4 partition-strided
    SBUF->SBUF DMAs from pooled h1.
  - conv3: M=128ch, K=(64ci x 2dy)+64ci; rhs via 3 SBUF DMAs.
  - maxpools fused per conv chunk on DVE (x: stride-2 pairs, y: partition
    block pairs); conv3 relu accumulates straight into q via ACT accum_out.
  - retrieval: sim in fp32, top-5 threshold via 5x(reduce_max + mask),
    masked stable softmax, mem = values^T @ e^T; decoder collapsed to 3
    dense matmuls (host-built dense equivalents of ConvT 1x1->2x2->4x4
    and the final 3x3 conv).
"""

import os
import sys
import numpy as np

for _p in ("/opt/trn_rl_repo", "/root/.axon_site/_ro/trn_rl_repo"):
    if os.path.isdir(_p) and _p not in sys.path:
        sys.path.append(_p)

import concourse.bass as bass
import concourse.mybir as mybir
import concourse.tile as tile
from concourse import bacc
from concourse.bass_utils import run_bass_kernel_spmd

F32 = mybir.dt.float32
BF16 = mybir.dt.bfloat16

N_CORES = 8
B = 128
H = 256

# STAGE: "bf16" (hi/lo-split weights, bf16 activations) | "f32" (all fp32)
STAGE = os.environ.get("K_STAGE", "bf16")
NSPLIT = 2 if STAGE == "bf16" else 1

AluOp = mybir.AluOpType
ActFn = mybir.ActivationFunctionType
AxX = mybir.AxisListType.X


def _sdt():
    return BF16 if STAGE == "bf16" else F32


def _np_sdt():
    if STAGE == "bf16":
        import ml_dtypes
        return ml_dtypes.bfloat16
    return np.float32


# ---------------------------------------------------------------------------
# host-side constant construction
# ---------------------------------------------------------------------------
def _split_last2(w):
    """[..., P, N] fp32 -> [..., 2, P, N] (hi, lo) in staging dtype, stacked
    on a new axis before the partition axis... callers reshape as needed."""
    sdt = _np_sdt()
    if NSPLIT == 1:
        return w.astype(np.float32)[..., None, :, :].swapaxes(0, 0)
    hi = w.astype(sdt).astype(np.float32)
    lo = (w - hi)
    return np.stack([hi.astype(sdt), lo.astype(sdt)], axis=0)


def _pack2(m):  # [256, N] -> [128, 2, N]
    return np.ascontiguousarray(m.reshape(2, 128, -1).transpose(1, 0, 2))


def _fft_consts():
    k = np.arange(H)
    F = np.exp(-2j * np.pi * np.outer(k, k) / H) / 16.0
    G = np.roll(F, H // 2, axis=0)
    GT = G.T.copy()
    out = {}
    for name, m in (("gre", GT.real), ("gim", GT.imag), ("gimn", -GT.imag)):
        m = m.astype(np.float32)
        if NSPLIT == 1:
            out[name] = _pack2(m)[:, :, None, :]  # [128, 2, 1, 256] f32
        else:
            sdt = _np_sdt()
            hi32 = m.astype(sdt).astype(np.float32)
            lo = (m - hi32).astype(sdt)
            hi = m.astype(sdt)
            # [128, 2, 2, 256]: (part-of-256-rows, half, split, col)
            out[name] = np.stack([_pack2(hi), _pack2(lo)], axis=2)
    return out


T1ORD = (0, 2, 1, 3)  # conv1 M block -> strip row offset t; so that y-pool
# pairs (t0,t1),(t2,t3) become max(partitions 0:64, partitions 64:128)


def _conv1_lhsT(we1):
    # K = (dx, j) packed on 18 partitions, p = dx*6 + j (dx-major so each
    # strip1 fill DMA writes a contiguous partition block)
    out = np.zeros((18, 1, 128), np.float32)
    for dx in range(3):
        for m in range(128):
            t, co = T1ORD[m // 32], m % 32
            for j in range(6):
                if 0 <= j - t <= 2:
                    out[dx * 6 + j, 0, m] = we1[co, 0, j - t, dx]
    return out


def _conv2_lhsT(we2):
    # K layout j-major: k = j*32 + ci (each strip2 fill writes a contiguous
    # partition block, keeping the CoreSim race detector happy)
    out = np.zeros((128, 3, 128), np.float32)
    for dx in range(3):
        for m in range(128):
            t, co = m // 64, m % 64
            for k in range(128):
                ci, j = k % 32, k // 32
                if 0 <= j - t <= 2:
                    out[k, dx, m] = we2[co, ci, j - t, dx]
    return out


def _conv3_lhsT(we3):
    # K layout d-major: k = d*64 + ci
    A = np.zeros((128, 3, 128), np.float32)
    Bm = np.zeros((64, 3, 128), np.float32)
    for dx in range(3):
        for k in range(128):
            ci, d = k % 64, k // 64
            A[k, dx, :] = we3[:, ci, d, dx]
        for ci in range(64):
            Bm[ci, dx, :] = we3[:, ci, 2, dx]
    return A, Bm


def _wsplit(w):
    """[P, D, N] -> [P, D, NSPLIT, N] staging dtype (hi, lo)."""
    sdt = _np_sdt()
    if NSPLIT == 1:
        return w[:, :, None, :].astype(np.float32)
    hi32 = w.astype(sdt).astype(np.float32)
    lo = (w - hi32).astype(sdt)
    return np.stack([w.astype(sdt), lo], axis=2)


def _decoder_mats(wd1, bd1, wd2, bd2, wd3, bd3):
    W1 = np.zeros((128, 256), np.float32)
    for c in range(64):
        for i in range(2):
            for j in range(2):
                W1[:, c * 4 + i * 2 + j] = wd1[:, c, i + 1, j + 1]
    b1 = np.repeat(bd1, 4).astype(np.float32)

    W2 = np.zeros((256, 512), np.float32)
    for c in range(64):
        for ii in range(2):
            for jj in range(2):
                f = c * 4 + ii * 2 + jj
                for c2 in range(32):
                    for y in range(4):
                        ky = y + 1 - 2 * ii
                        if not (0 <= ky <= 3):
                            continue
                        for x in range(4):
                            kx = x + 1 - 2 * jj
                            if 0 <= kx <= 3:
                                W2[f, c2 * 16 + y * 4 + x] = wd2[c, c2, ky, kx]
    b2 = np.repeat(bd2, 16).astype(np.float32)

    W3 = np.zeros((512, 16), np.float32)
    for c2 in range(32):
        for y in range(4):
            for x in range(4):
                g = c2 * 16 + y * 4 + x
                for oy in range(4):
                    ky = y - oy + 1
                    if not (0 <= ky <= 2):
                        continue
                    for ox in range(4):
                        kx = x - ox + 1
                        if 0 <= kx <= 2:
                            W3[g, oy * 4 + ox] = wd3[0, c2, ky, kx]
    b3 = np.full((16,), float(np.asarray(bd3).reshape(-1)[0]), np.float32)
    return W1, b1, W2, b2, W3, b3


def _host_consts(inputs):
    w3a, w3b = _conv3_lhsT(np.asarray(inputs["we3"], np.float32))
    W1, b1, W2, b2, W3, b3 = _decoder_mats(
        np.asarray(inputs["wd1"], np.float32), np.asarray(inputs["bd1"], np.float32),
        np.asarray(inputs["wd2"], np.float32), np.asarray(inputs["bd2"], np.float32),
        np.asarray(inputs["wd3"], np.float32), np.asarray(inputs["bd3"], np.float32))

    keys = np.asarray(inputs["keys"], np.float32)
    values = np.asarray(inputs["values"], np.float32)
    keys_p = np.ones((512, 128), np.float32)
    keys_p[:400] = keys
    values_p = np.zeros((512, 128), np.float32)
    values_p[:400] = values

    c = dict(_fft_consts())
    c.update({
        "w1l": _wsplit(_conv1_lhsT(np.asarray(inputs["we1"], np.float32))),
        "w2l": _wsplit(_conv2_lhsT(np.asarray(inputs["we2"], np.float32))),
        "w3a": _wsplit(w3a), "w3b": _wsplit(w3b),
        "cb1": np.tile(np.asarray(inputs["be1"], np.float32), 4).reshape(128, 1),
        "cb2": np.tile(np.asarray(inputs["be2"], np.float32), 2).reshape(128, 1),
        "cb3": np.asarray(inputs["be3"], np.float32).reshape(128, 1),
        "keys": np.ascontiguousarray(keys_p.reshape(4, 128, 128).transpose(1, 0, 2)),
        "vals": np.ascontiguousarray(values_p.reshape(4, 128, 128).transpose(1, 0, 2)),
        "ident": np.eye(128, dtype=np.float32),
        "w1d": W1,
        "w2d": np.ascontiguousarray(W2.reshape(2, 128, 4, 128).transpose(1, 0, 2, 3)),
        "w3d": np.ascontiguousarray(W3.reshape(4, 128, 16).transpose(1, 0, 2)),
        "b1d": np.ascontiguousarray(b1.reshape(2, 128).T),
        "b2d": np.ascontiguousarray(b2.reshape(4, 128).T),
        "b3row": b3.reshape(1, 16),
        "ones1": np.ones((1, 16), np.float32),
    })
    return c


def _const_specs():
    s = "stage"
    return {
        "gre": ([128, 2, NSPLIT, 256], s), "gim": ([128, 2, NSPLIT, 256], s),
        "gimn": ([128, 2, NSPLIT, 256], s),
        "w1l": ([18, 1, NSPLIT, 128], s), "w2l": ([128, 3, NSPLIT, 128], s),
        "w3a": ([128, 3, NSPLIT, 128], s), "w3b": ([64, 3, NSPLIT, 128], s),
        "cb1": ([128, 1], "f32"), "cb2": ([128, 1], "f32"), "cb3": ([128, 1], "f32"),
        "keys": ([128, 4, 128], "f32"), "vals": ([128, 4, 128], "f32"),
        "ident": ([128, 128], "f32"),
        "w1d": ([128, 256], "f32"), "w2d": ([128, 2, 4, 128], "f32"),
        "w3d": ([128, 4, 16], "f32"),
        "b1d": ([128, 2], "f32"), "b2d": ([128, 4], "f32"),
        "b3row": ([1, 16], "f32"), "ones1": ([1, 16], "f32"),
    }


def mk(t, poff, pstep, pcount, fdims, foff=0):
    """Manual AP on tile t (element units; partition pitch from the tile AP)."""
    pitch = t.ap[0][0]
    dims = [[pstep * pitch, pcount]] + [list(d) for d in fdims]
    return bass.AP(t.tensor, t.offset + poff * pitch + foff, dims)


def dramap(t, off, dims):
    return bass.AP(t.tensor, t.offset + off, [list(d) for d in dims])


# ---------------------------------------------------------------------------
# kernel builder
# ---------------------------------------------------------------------------
def build_nc(b_loc=16):
    sdt = _sdt()
    fft_in_dt = BF16 if STAGE == "bf16" else F32
    nc = bacc.Bacc("TRN2", target_bir_lowering=False, debug=False)

    x_in = nc.dram_tensor("x_in", [b_loc, 128, 2, 256], fft_in_dt,
                          kind="ExternalInput")
    out_d = nc.dram_tensor("out", [b_loc, 16], F32, kind="ExternalOutput")
    const_d = {}
    for name, (shape, kind) in _const_specs().items():
        dt_ = _sdt() if kind == "stage" else F32
        const_d[name] = nc.dram_tensor(name, shape, dt_, kind="ExternalInput")

    with tile.TileContext(nc) as tc:
        from contextlib import ExitStack
        with ExitStack() as ctx:
            cpool = ctx.enter_context(tc.tile_pool(name="consts", bufs=1))
            spool = ctx.enter_context(tc.tile_pool(name="stage", bufs=1))
            wpool = ctx.enter_context(tc.tile_pool(name="work", bufs=2))
            rpool = ctx.enter_context(tc.tile_pool(name="ret", bufs=1))
            dpool = ctx.enter_context(tc.tile_pool(name="dram", bufs=1, space="DRAM"))
            fftps = ctx.enter_context(tc.tile_pool(name="fftps", bufs=4, space="PSUM"))
            convps = ctx.enter_context(tc.tile_pool(name="convps", bufs=2, space="PSUM"))
            miscps = ctx.enter_context(tc.tile_pool(name="miscps", bufs=2, space="PSUM"))

            cs = {}
            for name, (shape, kind) in _const_specs().items():
                dt_ = _sdt() if kind == "stage" else F32
                t = cpool.tile(shape, dt_, name=f"c_{name}")
                nc.sync.dma_start(out=t, in_=const_d[name].ap())
                cs[name] = t

            # fixed stage buffers
            strip1 = spool.tile([18, 32, 256], sdt, name="strip1")
            strip2 = spool.tile([128, 64, 130], sdt, name="strip2")
            strip3a = spool.tile([128, 64, 66], sdt, name="strip3a")
            strip3b = spool.tile([64, 64, 66], sdt, name="strip3b")
            # h1X: pooled conv1, partitions (parity g, co): p = 32g + co,
            # free (s = y'//2, x')
            h1X = spool.tile([64, 64, 128], sdt, name="h1X")
            h2buf = spool.tile([64, 64, 64], sdt, name="h2buf")
            xpooled1 = spool.tile([128, 64, 128], sdt, name="xpooled1")
            xpooled2 = spool.tile([128, 64, 64], sdt, name="xpooled2")
            xpB1 = spool.tile([64, 64, 128], sdt, name="xpB1")
            xpB2 = spool.tile([64, 64, 64], sdt, name="xpB2")
            qT = spool.tile([128, b_loc], F32, name="qT")
            xmd = dpool.tile([258, 258], sdt, name="xmd")

            for t in (strip1, strip2, strip3a, strip3b):
                nc.vector.memset(t, 0.0)
            zrow = cpool.tile([1, 2, 258], sdt, name="zrow")
            nc.vector.memset(zrow, 0.0)
            nc.sync.dma_start(  # pad rows 0, 257
                out=dramap(xmd, 0, [[1, 1], [257 * 258, 2], [1, 258]]),
                in_=zrow)
            for col in (0, 257):  # pad cols 0, 257
                nc.sync.dma_start(
                    out=dramap(xmd, col, [[1, 1], [258, 258]]),
                    in_=zrow[0:1, 0, :])

            # ---- key normalization -> knT [128, 400] (fp32)
            knT = rpool.tile([128, 400], F32, name="knT")
            ksq = rpool.tile([128, 4, 128], F32, name="ksq")
            nc.vector.tensor_mul(ksq, cs["keys"], cs["keys"])
            kss = rpool.tile([128, 4], F32, name="kss")
            nc.vector.reduce_sum(kss, ksq, axis=AxX)
            knm = rpool.tile([128, 4], F32, name="knm")
            nc.scalar.sqrt(knm, kss)
            nc.vector.tensor_scalar_max(knm, knm, 1e-12)
            kri = rpool.tile([128, 4], F32, name="kri")
            nc.vector.reciprocal(kri, knm)
            knrm = rpool.tile([128, 4, 128], F32, name="knrm")
            for c in range(4):
                nc.vector.tensor_scalar_mul(
                    knrm[:, c, :], cs["keys"][:, c, :], kri[:, c:c + 1])
            for c in range(4):
                pc = 128 if c < 3 else 16
                tp = miscps.tile([128, 128], F32, name="tp_kn", tag="misc")
                nc.tensor.transpose(
                    tp[:, :pc], knrm[:pc, c, :], cs["ident"][:pc, :pc])
                nc.scalar.copy(knT[:, c * 128:c * 128 + pc], tp[:, :pc])

            # ---- per-image pipeline (K_REP>1: timing amplification loop)
            rep = int(os.environ.get("K_REP", "1"))
            loop_cm = tc.For_i(0, rep, 1) if rep > 1 else None
            if loop_cm is not None:
                loop_cm.__enter__()
            for img in range(b_loc):
                x_sb = wpool.tile([128, 2, 256], fft_in_dt, name="x_sb",
                                  tag="x_sb")
                nc.sync.dma_start(
                    out=x_sb,
                    in_=dramap(x_in.ap(), img * 65536,
                               [[512, 128], [256, 2], [1, 256]]))

                # FFT step 1: yT = x^T @ GT (re, im)
                yts = {}
                for nm, rt in (("re", "gre"), ("im", "gim")):
                    for mt in range(2):
                        ps = fftps.tile([128, 256], F32, name="ps_yt", tag="fft")
                        n_mm = 2 * NSPLIT
                        i = 0
                        for kt in range(2):
                            for sp in range(NSPLIT):
                                nc.tensor.matmul(
                                    ps,
                                    x_sb[:, kt, mt * 128:(mt + 1) * 128],
                                    cs[rt][:, kt, sp, :],
                                    start=(i == 0), stop=(i == n_mm - 1))
                                i += 1
                        sb = wpool.tile([128, 256], fft_in_dt, name=f"yt{nm}{mt}",
                                        tag=f"yt{nm}{mt}")
                        nc.scalar.copy(sb, ps)
                        yts[(nm, mt)] = sb

                # FFT step 2 + magnitude
                xm_sb = wpool.tile([128, 2, 256], sdt, name="xm_sb", tag="xm_sb")
                for mt in range(2):
                    zre = fftps.tile([128, 256], F32, name="ps_zre", tag="fft")
                    zim = fftps.tile([128, 256], F32, name="ps_zim", tag="fft")
                    for out_ps, combos in (
                        (zre, [("re", "gre"), ("im", "gimn")]),
                        (zim, [("re", "gim"), ("im", "gre")]),
                    ):
                        n_mm = 4 * NSPLIT
                        i = 0
                        for nm, rt in combos:
                            for kt in range(2):
                                for sp in range(NSPLIT):
                                    nc.tensor.matmul(
                                        out_ps,
                                        yts[(nm, kt)][:, mt * 128:(mt + 1) * 128],
                                        cs[rt][:, kt, sp, :],
                                        start=(i == 0), stop=(i == n_mm - 1))
                                    i += 1
                    t1 = wpool.tile([128, 256], F32, name="mag1", tag="mag1")
                    t2 = wpool.tile([128, 256], F32, name="mag2", tag="mag2")
                    nc.scalar.square(t1, zre)
                    nc.scalar.square(t2, zim)
                    nc.vector.tensor_add(t1, t1, t2)
                    nc.scalar.sqrt(xm_sb[:, mt, :], t1)

                # xm -> DRAM bounce (rows 1..256)
                nc.sync.dma_start(
                    out=dramap(xmd, 258 + 1,
                               [[258, 128], [128 * 258, 2], [1, 256]]),
                    in_=xm_sb)

                # ---- conv1 over 2 half-images
                for hs in range(2):
                    for dx in range(3):
                        nc.sync.dma_start(
                            out=mk(strip1, 6 * dx, 1, 6, [[256, 32], [1, 256]]),
                            in_=dramap(xmd, 33024 * hs + dx,
                                       [[258, 6], [1032, 32], [1, 256]]))
                    for ch in range(16):
                        sg = 32 * hs + 2 * ch
                        ps = convps.tile([128, 512], F32, name="c1ps", tag="conv")
                        for sp in range(NSPLIT):
                            nc.tensor.matmul(
                                ps, cs["w1l"][:, 0, sp, :],
                                strip1[:, 2 * ch:2 * ch + 2, :],
                                start=(sp == 0), stop=(sp == NSPLIT - 1))
                        rt = wpool.tile([128, 2, 256], sdt, name="rt1", tag="rt1",
                                        bufs=3)
                        nc.scalar.activation(rt, ps, ActFn.Relu,
                                             bias=cs["cb1"][:, 0:1])
                        nc.vector.tensor_max(
                            mk(xpooled1, 0, 1, 128, [[128, 2], [1, 128]],
                               sg * 128),
                            mk(rt, 0, 1, 128, [[256, 2], [2, 128]], 0),
                            mk(rt, 0, 1, 128, [[256, 2], [2, 128]], 1))

                # y-pool: blocks are [t0, t2 | t1, t3]; DMA-align the upper
                # half to base partition 0, then one equal-base DVE max
                nc.sync.dma_start(
                    out=xpB1,
                    in_=mk(xpooled1, 64, 1, 64, [[128, 64], [1, 128]]))
                nc.vector.tensor_max(
                    h1X, mk(xpooled1, 0, 1, 64, [[128, 64], [1, 128]]), xpB1)

                # ---- conv2 staging: 4 SBUF DMAs from parity-split h1X
                # slot (ci, j, s2) holds h1 row r = 2*s2 + j - 1
                fills = [(0, 1, 63, 1, 0), (1, 0, 64, 0, 0),
                         (2, 0, 64, 1, 0), (3, 0, 63, 0, 1)]
                for j, s2o, ns, g, s0 in fills:
                    nc.sync.dma_start(
                        out=mk(strip2, 32 * j, 1, 32, [[130, ns], [1, 128]],
                               s2o * 130 + 1),
                        in_=mk(h1X, 32 * g, 1, 32, [[128, ns], [1, 128]],
                               s0 * 128))
                for ch in range(16):
                    ps = convps.tile([128, 512], F32, name="c2ps", tag="conv")
                    i = 0
                    for dx in range(3):
                        for sp in range(NSPLIT):
                            nc.tensor.matmul(
                                ps, cs["w2l"][:, dx, sp, :],
                                mk(strip2, 0, 1, 128, [[130, 4], [1, 128]],
                                   4 * ch * 130 + dx),
                                start=(i == 0), stop=(i == 3 * NSPLIT - 1))
                            i += 1
                    rt2 = wpool.tile([128, 4, 128], sdt, name="rt2", tag="rt2",
                                     bufs=3)
                    nc.scalar.activation(rt2, ps, ActFn.Relu,
                                         bias=cs["cb2"][:, 0:1])
                    nc.vector.tensor_max(
                        mk(xpooled2, 0, 1, 128, [[64, 4], [1, 64]],
                           4 * ch * 64),
                        mk(rt2, 0, 1, 128, [[128, 4], [2, 64]], 0),
                        mk(rt2, 0, 1, 128, [[128, 4], [2, 64]], 1))
                nc.sync.dma_start(
                    out=xpB2,
                    in_=mk(xpooled2, 64, 1, 64, [[64, 64], [1, 64]]))
                nc.vector.tensor_max(
                    h2buf, mk(xpooled2, 0, 1, 64, [[64, 64], [1, 64]]), xpB2)

                # ---- conv3 staging
                nc.sync.dma_start(
                    out=mk(strip3a, 0, 1, 64, [[66, 63], [1, 64]], 66 + 1),
                    in_=mk(h2buf, 0, 1, 64, [[64, 63], [1, 64]], 0))
                nc.sync.dma_start(
                    out=mk(strip3a, 64, 1, 64, [[66, 64], [1, 64]], 1),
                    in_=mk(h2buf, 0, 1, 64, [[64, 64], [1, 64]], 0))
                nc.sync.dma_start(
                    out=mk(strip3b, 0, 1, 64, [[66, 63], [1, 64]], 1),
                    in_=mk(h2buf, 0, 1, 64, [[64, 63], [1, 64]], 64))

                qacc = wpool.tile([128, 8], F32, name="qacc", tag="qacc")
                for ch in range(8):
                    ps = convps.tile([128, 512], F32, name="c3ps", tag="conv")
                    n_mm = 6 * NSPLIT
                    i = 0
                    for dx in range(3):
                        for w_, st3, pc in (("w3a", strip3a, 128),
                                            ("w3b", strip3b, 64)):
                            for sp in range(NSPLIT):
                                nc.tensor.matmul(
                                    ps, cs[w_][:, dx, sp, :],
                                    mk(st3, 0, 1, pc, [[66, 8], [1, 64]],
                                       8 * ch * 66 + dx),
                                    start=(i == 0), stop=(i == n_mm - 1))
                                i += 1
                    scr = wpool.tile([128, 512], F32, name="scr3", tag="scr3",
                                     bufs=2)
                    nc.scalar.activation(scr, ps, ActFn.Relu,
                                         bias=cs["cb3"][:, 0:1],
                                         accum_out=qacc[:, ch:ch + 1])
                nc.vector.reduce_sum(qT[:, img:img + 1], qacc, axis=AxX)

            # ---------------- retrieval (fp32) ----------------
            bl = b_loc
            simps = miscps.tile([bl, 400], F32, name="simps", tag="misc")
            nc.tensor.matmul(simps, qT, knT, start=True, stop=True)
            gram = miscps.tile([bl, bl], F32, name="gram", tag="misc")
            nc.tensor.matmul(gram, qT, qT, start=True, stop=True)
            gd = rpool.tile([bl, bl], F32, name="gd")
            nc.vector.tensor_mul(gd, gram, cs["ident"][:bl, :bl])
            q2 = rpool.tile([bl, 1], F32, name="q2")
            nc.vector.reduce_sum(q2, gd, axis=AxX)
            qn = rpool.tile([bl, 1], F32, name="qn")
            nc.scalar.sqrt(qn, q2)
            nc.vector.tensor_scalar_max(qn, qn, 1e-12)
            rq = rpool.tile([bl, 1], F32, name="rq")
            nc.vector.reciprocal(rq, qn)
            sim = rpool.tile([bl, 400], F32, name="sim")
            nc.vector.tensor_scalar_mul(sim, simps, rq[:, 0:1])

            cur = rpool.tile([bl, 400], F32, name="cur")
            nc.vector.tensor_copy(cur, sim)
            m1 = rpool.tile([bl, 1], F32, name="m1")
            nc.vector.reduce_max(m1, sim, axis=AxX)
            msk = rpool.tile([bl, 400], F32, name="msk")
            mk_ = m1
            for it in range(4):
                nc.vector.tensor_scalar(msk, cur, mk_[:, 0:1], None,
                                        op0=AluOp.is_ge)
                nc.vector.scalar_tensor_tensor(cur, msk, -1e30, cur,
                                               op0=AluOp.mult, op1=AluOp.add)
                nm_ = rpool.tile([bl, 1], F32, name=f"mk{it}")
                nc.vector.reduce_max(nm_, cur, axis=AxX)
                mk_ = nm_
            m5 = mk_
            nc.vector.tensor_scalar(msk, sim, m5[:, 0:1], None, op0=AluOp.is_ge)
            m1n = rpool.tile([bl, 1], F32, name="m1n")
            nc.vector.tensor_scalar_mul(m1n, m1, -1.0)
            es = rpool.tile([bl, 400], F32, name="es")
            nc.scalar.activation(es, sim, ActFn.Exp, bias=m1n[:, 0:1])
            ew = rpool.tile([bl, 400], F32, name="ew")
            nc.vector.tensor_mul(ew, es, msk)
            zs = rpool.tile([bl, 1], F32, name="zs")
            nc.vector.reduce_sum(zs, ew, axis=AxX)
            rz = rpool.tile([bl, 1], F32, name="rz")
            nc.vector.reciprocal(rz, zs)
            nc.vector.tensor_scalar_mul(ew, ew, rz[:, 0:1])

            eT = rpool.tile([128, 4, bl], F32, name="eT")
            for c in range(4):
                pc = 128 if c < 3 else 16
                tp = miscps.tile([128, bl], F32, name="tp_e", tag="misc")
                nc.tensor.transpose(tp[:pc, :], ew[:, c * 128:c * 128 + pc],
                                    cs["ident"][:bl, :bl])
                nc.scalar.copy(eT[:pc, c, :], tp[:pc, :])

            memps = miscps.tile([128, bl], F32, name="memps", tag="misc")
            for c in range(4):
                pc = 128 if c < 3 else 16
                nc.tensor.matmul(memps, cs["vals"][:pc, c, :], eT[:pc, c, :],
                                 start=(c == 0), stop=(c == 3))
            memT = rpool.tile([128, bl], F32, name="memT")
            nc.scalar.copy(memT, memps)

            h1T = rpool.tile([128, 2, bl], F32, name="h1T")
            for mt in range(2):
                ps = miscps.tile([128, bl], F32, name="d1ps", tag="misc")
                nc.tensor.matmul(ps, cs["w1d"][:, mt * 128:(mt + 1) * 128],
                                 memT, start=True, stop=True)
                nc.scalar.activation(h1T[:, mt, :], ps, ActFn.Relu,
                                     bias=cs["b1d"][:, mt:mt + 1])
            h2T = rpool.tile([128, 4, bl], F32, name="h2T")
            for mt in range(4):
                ps = miscps.tile([128, bl], F32, name="d2ps", tag="misc")
                for kt in range(2):
                    nc.tensor.matmul(ps, cs["w2d"][:, kt, mt, :], h1T[:, kt, :],
                                     start=(kt == 0), stop=(kt == 1))
                nc.scalar.activation(h2T[:, mt, :], ps, ActFn.Relu,
                                     bias=cs["b2d"][:, mt:mt + 1])
            ops = miscps.tile([bl, 16], F32, name="outps", tag="misc")
            for c in range(4):
                nc.tensor.matmul(ops, h2T[:, c, :], cs["w3d"][:, c, :],
                                 start=(c == 0), stop=False)
            nc.tensor.matmul(ops, cs["ones1"][:, :bl], cs["b3row"],
                             start=False, stop=True)
            out_sb = rpool.tile([bl, 16], F32, name="out_sb")
            nc.scalar.copy(out_sb, ops)
            nc.sync.dma_start(out=out_d.ap(), in_=out_sb)
            if loop_cm is not None:
                loop_cm.__exit__(None, None, None)

    nc.compile()
    return nc


# ---------------------------------------------------------------------------
# host entry
# ---------------------------------------------------------------------------
_NC_CACHE = {}


def _get_nc(b_loc):
    key = (b_loc, STAGE, os.environ.get("K_REP", "1"))
    if key not in _NC_CACHE:
        _NC_CACHE[key] = build_nc(b_loc)
    return _NC_CACHE[key]


def _pack_x(x_shard):
    b = x_shard.shape[0]
    xr = np.ascontiguousarray(
        x_shard.reshape(b, 2, 128, 256).transpose(0, 2, 1, 3)).astype(np.float32)
    return xr.astype(_np_sdt())


def kernel(**inputs):
    x = np.asarray(inputs["x"], np.float32)
    # jnp.fft.fftshift also shifts the batch axis: output b uses x[(b+64)%128]
    xp = np.roll(x, -64, axis=0)
    consts = _host_consts(inputs)

    b_loc = B // N_CORES
    nc = _get_nc(b_loc)

    in_maps = []
    for c in range(N_CORES):
        m = dict(consts)
        m["x_in"] = _pack_x(xp[c * b_loc:(c + 1) * b_loc])
        in_maps.append(m)

    kwargs = {}
    if os.environ.get("K_TRACE"):
        kwargs["trace"] = True
        try:  # dev-only: register the axon NTFF hook missing from the image
            import types
            import antenv
            if not hasattr(antenv, "axon_hooks"):
                import trn_agent_boot.trn_boot as _tb
                _hook = _tb._ntff_profile_via_ctypes("/opt/axon/libaxon_pjrt.so")
                _m = types.ModuleType("antenv.axon_hooks")
                _hh = [_hook]
                _m.set_axon_ntff_profile_hook = lambda h: _hh.__setitem__(0, h)
                _m.get_axon_ntff_profile_hook = lambda: _hh[0]
                sys.modules["antenv.axon_hooks"] = _m
                antenv.axon_hooks = _m
        except Exception as e:
            print("trace hook shim failed:", e)
    res = run_bass_kernel_spmd(nc, in_maps, core_ids=list(range(N_CORES)),
                               **kwargs)
    global LAST_RESULTS
    LAST_RESULTS = res
    out = np.concatenate([r["out"] for r in res.results], axis=0)
    return out.reshape(B, 1, 4, 4).astype(np.float32)


LAST_RESULTS = None


if __name__ == "__main__":
    build_nc(int(os.environ.get("K_BLOC", "1")))
    print("built ok")



# revision 17
# speedup vs baseline: 1.6815x; 1.6815x over previous
"""Trainium2 Bass kernel for nn_FFTMemAutoEncoderBranch (retrieval_knn).

Data-parallel over batch: 8 cores x 16 images, no cross-core communication.

v2 numerics (validated on host emulation vs the fp32 reference: 0/128 top-5
flips, ~5e-7 output rel err):
  - activations fp16, PSUM fp32
  - DFT matrices G single-pass fp16 (emulation shows G rounding washes out)
  - conv weights fp16 hi/lo split (single-pass fp16 flips top-5 retrieval)
  - retrieval + decoder fp32

v2 structure (vs v1: ~2x fewer matmul rows, pool-before-relu, pitch-padded
strips with contiguous fills, software-pipelined FFT one image ahead):
  - FFT2 as DFT matmuls, G single fp16: 24 matmuls/image N=256.
  - conv1: hi/lo folded into K (K=36 = 2sp x 18taps, strip data duplicated
    across partition halves by the fill DMA) -> ONE matmul per chunk.
  - conv2: K=128 (4j x 32ci), 6 accs (3dx x 2sp); strip2 pitch-129 x-wrap
    padding -> 4 contiguous SBUF fill DMAs from parity-split h1X.
  - conv3: per-tap K=128 (2sp x 64ci, acts duplicated) -> 9 accs; strip3
    pitch-65 x-wrap, one dup-fill DMA from h2buf.
  - pools: x-pool via DVE pool_max straight off PSUM (f32), then ACT
    bias+relu+fp16-cast at half size; y-pool via DMA align + tensor_max.
  - strip fills issued on the Pool-engine (SWDGE) queue, rest on sync.
"""

import os
import sys
import numpy as np

for _p in ("/opt/trn_rl_repo", "/root/.axon_site/_ro/trn_rl_repo"):
    if os.path.isdir(_p) and _p not in sys.path:
        sys.path.append(_p)

import concourse.bass as bass
import concourse.mybir as mybir
import concourse.tile as tile
from concourse import bacc
from concourse.bass_utils import run_bass_kernel_spmd

F32 = mybir.dt.float32
F16 = mybir.dt.float16

N_CORES = 8
B = 128
H = 256

AluOp = mybir.AluOpType
ActFn = mybir.ActivationFunctionType
AxX = mybir.AxisListType.X

T1ORD = (0, 2, 1, 3)  # conv1 M-block order so y-pool pairs are (p, p+64)


# ---------------------------------------------------------------------------
# host-side constant construction
# ---------------------------------------------------------------------------
def _sp(w):
    """fp32 -> (hi, lo) fp16 split parts, returned as fp32 arrays."""
    w = np.asarray(w, np.float32)
    hi = w.astype(np.float16).astype(np.float32)
    lo = (w - hi)
    return hi, lo


def _pack2(m):  # [256, N] -> [128, 2, N]
    return np.ascontiguousarray(m.reshape(2, 128, -1).transpose(1, 0, 2))


def _fft_consts():
    k = np.arange(H)
    F = np.exp(-2j * np.pi * np.outer(k, k) / H) / 16.0
    G = np.roll(F, H // 2, axis=0)
    GT = G.T.copy()
    out = {}
    for name, m in (("gre", GT.real), ("gim", GT.imag), ("gimn", -GT.imag)):
        out[name] = _pack2(m.astype(np.float32)).astype(np.float16)
    return out


def _conv1_lhsT(we1):
    # K = (sp, dx, j): k = sp*18 + dx*6 + j; strip partitions 18..35 hold a
    # duplicate of 0..17 so hi+lo accumulate in one matmul.
    hi, lo = _sp(we1)
    out = np.zeros((36, 128), np.float32)
    for sp, w in ((0, hi), (1, lo)):
        for dx in range(3):
            for m in range(128):
                t, co = T1ORD[m // 32], m % 32
                for j in range(6):
                    if 0 <= j - t <= 2:
                        out[sp * 18 + dx * 6 + j, m] = w[co, 0, j - t, dx]
    return out.astype(np.float16)


def _conv2_lhsT(we2):
    # k = j*32 + ci (4 y-offsets j packed in K); [128, 3dx, 2sp, 128m]
    hi, lo = _sp(we2)
    out = np.zeros((128, 3, 2, 128), np.float32)
    for sp, w in ((0, hi), (1, lo)):
        for dx in range(3):
            for m in range(128):
                t, co = m // 64, m % 64
                for k in range(128):
                    ci, j = k % 32, k // 32
                    if 0 <= j - t <= 2:
                        out[k, dx, sp, m] = w[co, ci, j - t, dx]
    return out.astype(np.float16)


def _conv3_lhsT(we3):
    # per-tap K = (sp, ci): k = sp*64 + ci; taps t9 = dy*3+dx; [128, 9, 128]
    hi, lo = _sp(we3)
    out = np.zeros((128, 9, 128), np.float32)
    for sp, w in ((0, hi), (1, lo)):
        for dy in range(3):
            for dx in range(3):
                for ci in range(64):
                    out[sp * 64 + ci, dy * 3 + dx, :] = w[:, ci, dy, dx]
    return out.astype(np.float16)


def _decoder_mats(wd1, bd1, wd2, bd2, wd3, bd3):
    W1 = np.zeros((128, 256), np.float32)
    for c in range(64):
        for i in range(2):
            for j in range(2):
                W1[:, c * 4 + i * 2 + j] = wd1[:, c, i + 1, j + 1]
    b1 = np.repeat(bd1, 4).astype(np.float32)

    W2 = np.zeros((256, 512), np.float32)
    for c in range(64):
        for ii in range(2):
            for jj in range(2):
                f = c * 4 + ii * 2 + jj
                for c2 in range(32):
                    for y in range(4):
                        ky = y + 1 - 2 * ii
                        if not (0 <= ky <= 3):
                            continue
                        for x in range(4):
                            kx = x + 1 - 2 * jj
                            if 0 <= kx <= 3:
                                W2[f, c2 * 16 + y * 4 + x] = wd2[c, c2, ky, kx]
    b2 = np.repeat(bd2, 16).astype(np.float32)

    W3 = np.zeros((512, 16), np.float32)
    for c2 in range(32):
        for y in range(4):
            for x in range(4):
                g = c2 * 16 + y * 4 + x
                for oy in range(4):
                    ky = y - oy + 1
                    if not (0 <= ky <= 2):
                        continue
                    for ox in range(4):
                        kx = x - ox + 1
                        if 0 <= kx <= 2:
                            W3[g, oy * 4 + ox] = wd3[0, c2, ky, kx]
    b3 = np.full((16,), float(np.asarray(bd3).reshape(-1)[0]), np.float32)
    return W1, b1, W2, b2, W3, b3


def _host_consts(inputs):
    W1, b1, W2, b2, W3, b3 = _decoder_mats(
        np.asarray(inputs["wd1"], np.float32), np.asarray(inputs["bd1"], np.float32),
        np.asarray(inputs["wd2"], np.float32), np.asarray(inputs["bd2"], np.float32),
        np.asarray(inputs["wd3"], np.float32), np.asarray(inputs["bd3"], np.float32))

    keys = np.asarray(inputs["keys"], np.float32)
    values = np.asarray(inputs["values"], np.float32)
    keys_p = np.ones((512, 128), np.float32)
    keys_p[:400] = keys
    values_p = np.zeros((512, 128), np.float32)
    values_p[:400] = values

    c = dict(_fft_consts())
    c.update({
        "w1l": _conv1_lhsT(np.asarray(inputs["we1"], np.float32)),
        "w2l": _conv2_lhsT(np.asarray(inputs["we2"], np.float32)),
        "w3t": _conv3_lhsT(np.asarray(inputs["we3"], np.float32)),
        "cb1": np.tile(np.asarray(inputs["be1"], np.float32), 4).reshape(128, 1),
        "cb2": np.tile(np.asarray(inputs["be2"], np.float32), 2).reshape(128, 1),
        "cb3": np.asarray(inputs["be3"], np.float32).reshape(128, 1),
        "keys": np.ascontiguousarray(keys_p.reshape(4, 128, 128).transpose(1, 0, 2)),
        "vals": np.ascontiguousarray(values_p.reshape(4, 128, 128).transpose(1, 0, 2)),
        "ident": np.eye(128, dtype=np.float32),
        "w1d": W1,
        "w2d": np.ascontiguousarray(W2.reshape(2, 128, 4, 128).transpose(1, 0, 2, 3)),
        "w3d": np.ascontiguousarray(W3.reshape(4, 128, 16).transpose(1, 0, 2)),
        "b1d": np.ascontiguousarray(b1.reshape(2, 128).T),
        "b2d": np.ascontiguousarray(b2.reshape(4, 128).T),
        "b3row": b3.reshape(1, 16),
        "ones1": np.ones((1, 16), np.float32),
    })
    return c


def _const_specs():
    return {
        "gre": ([128, 2, 256], "f16"), "gim": ([128, 2, 256], "f16"),
        "gimn": ([128, 2, 256], "f16"),
        "w1l": ([36, 128], "f16"), "w2l": ([128, 3, 2, 128], "f16"),
        "w3t": ([128, 9, 128], "f16"),
        "cb1": ([128, 1], "f32"), "cb2": ([128, 1], "f32"), "cb3": ([128, 1], "f32"),
        "keys": ([128, 4, 128], "f32"), "vals": ([128, 4, 128], "f32"),
        "ident": ([128, 128], "f32"),
        "w1d": ([128, 256], "f32"), "w2d": ([128, 2, 4, 128], "f32"),
        "w3d": ([128, 4, 16], "f32"),
        "b1d": ([128, 2], "f32"), "b2d": ([128, 4], "f32"),
        "b3row": ([1, 16], "f32"), "ones1": ([1, 16], "f32"),
    }


def mk(t, poff, pstep, pcount, fdims, foff=0):
    """Manual AP on tile t (element units; partition pitch from the tile AP)."""
    pitch = t.ap[0][0]
    dims = [[pstep * pitch, pcount]] + [list(d) for d in fdims]
    return bass.AP(t.tensor, t.offset + poff * pitch + foff, dims)


def mk2(t, dims, foff=0):
    """AP with explicit raw dims (partition steps premultiplied by pitch)."""
    pitch = t.ap[0][0]
    out = []
    for step_p, step_f, count in dims:
        out.append([step_p * pitch + step_f, count])
    return bass.AP(t.tensor, t.offset + foff, out)


def dramap(t, off, dims):
    return bass.AP(t.tensor, t.offset + off, [list(d) for d in dims])


# ---------------------------------------------------------------------------
# kernel builder
# ---------------------------------------------------------------------------
def build_nc(b_loc=16):
    nc = bacc.Bacc("TRN2", target_bir_lowering=False, debug=False)

    x_in = nc.dram_tensor("x_in", [b_loc, 128, 2, 256], F16, kind="ExternalInput")
    out_d = nc.dram_tensor("out", [b_loc, 16], F32, kind="ExternalOutput")
    const_d = {}
    for name, (shape, kind) in _const_specs().items():
        dt_ = F16 if kind == "f16" else F32
        const_d[name] = nc.dram_tensor(name, shape, dt_, kind="ExternalInput")

    with tile.TileContext(nc) as tc:
        from contextlib import ExitStack
        with ExitStack() as ctx:
            cpool = ctx.enter_context(tc.tile_pool(name="consts", bufs=1))
            spool = ctx.enter_context(tc.tile_pool(name="stage", bufs=1))
            wpool = ctx.enter_context(tc.tile_pool(name="work", bufs=2))
            rpool = ctx.enter_context(tc.tile_pool(name="ret", bufs=1))
            dpool = ctx.enter_context(tc.tile_pool(name="dram", bufs=1, space="DRAM"))
            fftps = ctx.enter_context(tc.tile_pool(name="fftps", bufs=2, space="PSUM"))
            convps = ctx.enter_context(tc.tile_pool(name="convps", bufs=4, space="PSUM"))
            miscps = ctx.enter_context(tc.tile_pool(name="miscps", bufs=2, space="PSUM"))

            cs = {}
            for name, (shape, kind) in _const_specs().items():
                dt_ = F16 if kind == "f16" else F32
                t = cpool.tile(shape, dt_, name=f"c_{name}")
                nc.sync.dma_start(out=t, in_=const_d[name].ap())
                cs[name] = t

            # fixed stage buffers
            strip1 = spool.tile([36, 64, 256], F16, name="strip1")
            strip2 = spool.tile([128, 8386], F16, name="strip2")   # 1 + 65*129
            strip3 = spool.tile([128, 4291], F16, name="strip3")   # 1 + 66*65
            xpooled1 = spool.tile([128, 64, 129], F16, name="xpooled1")
            xpB1 = spool.tile([64, 8256], F16, name="xpB1")
            h1X = spool.tile([64, 8256], F16, name="h1X")
            xpooled2 = spool.tile([128, 64, 65], F16, name="xpooled2")
            xpB2 = spool.tile([64, 4160], F16, name="xpB2")
            h2buf = spool.tile([64, 4160], F16, name="h2buf")
            qT = spool.tile([128, b_loc], F32, name="qT")
            xmd = [dpool.tile([258, 258], F16, name=f"xmd{i}") for i in range(2)]

            for t in (strip2, strip3, xpooled1, xpooled2):
                nc.vector.memset(t, 0.0)
            zrow = cpool.tile([1, 2, 258], F16, name="zrow")
            nc.vector.memset(zrow, 0.0)
            for xm_d in xmd:
                nc.sync.dma_start(  # pad rows 0, 257
                    out=dramap(xm_d, 0, [[1, 1], [257 * 258, 2], [1, 258]]),
                    in_=zrow)
                for col in (0, 257):  # pad cols 0, 257
                    nc.sync.dma_start(
                        out=dramap(xm_d, col, [[1, 1], [258, 258]]),
                        in_=zrow[0:1, 0, :])

            # ---- key normalization -> knT [128, 400] (fp32)
            knT = rpool.tile([128, 400], F32, name="knT")
            ksq = rpool.tile([128, 4, 128], F32, name="ksq")
            nc.vector.tensor_mul(ksq, cs["keys"], cs["keys"])
            kss = rpool.tile([128, 4], F32, name="kss")
            nc.vector.reduce_sum(kss, ksq, axis=AxX)
            knm = rpool.tile([128, 4], F32, name="knm")
            nc.scalar.sqrt(knm, kss)
            nc.vector.tensor_scalar_max(knm, knm, 1e-12)
            kri = rpool.tile([128, 4], F32, name="kri")
            nc.vector.reciprocal(kri, knm)
            knrm = rpool.tile([128, 4, 128], F32, name="knrm")
            for c in range(4):
                nc.vector.tensor_scalar_mul(
                    knrm[:, c, :], cs["keys"][:, c, :], kri[:, c:c + 1])
            for c in range(4):
                pc = 128 if c < 3 else 16
                tp = miscps.tile([128, 128], F32, name="tp_kn", tag="misc")
                nc.tensor.transpose(
                    tp[:, :pc], knrm[:pc, c, :], cs["ident"][:pc, :pc])
                nc.scalar.copy(knT[:, c * 128:c * 128 + pc], tp[:, :pc])

            # ---------------- per-image pipeline ----------------
            def fft_stage(img):
                sb = strip1[img % 2]
                x_sb = wpool.tile([128, 2, 256], F16, name="x_sb", tag="x_sb")
                nc.sync.dma_start(
                    out=x_sb,
                    in_=dramap(x_in.ap(), img * 65536,
                               [[512, 128], [256, 2], [1, 256]]))

                # FFT step 1: yT = x^T @ GT (re, im)
                yts = {}
                for nm, rt in (("re", "gre"), ("im", "gim")):
                    for mt in range(2):
                        ps = fftps.tile([128, 256], F32, name="ps_yt", tag="fft")
                        for kt in range(2):
                            nc.tensor.matmul(
                                ps, x_sb[:, kt, mt * 128:(mt + 1) * 128],
                                cs[rt][:, kt, :],
                                start=(kt == 0), stop=(kt == 1))
                        sbt = wpool.tile([128, 256], F16, name=f"yt{nm}{mt}",
                                         tag=f"yt{nm}{mt}")
                        nc.scalar.copy(sbt, ps)
                        yts[(nm, mt)] = sbt

                # FFT step 2 + magnitude
                xm_sb = wpool.tile([128, 2, 256], F16, name="xm_sb", tag="xm_sb")
                for mt in range(2):
                    zre = fftps.tile([128, 256], F32, name="ps_zre", tag="fft")
                    zim = fftps.tile([128, 256], F32, name="ps_zim", tag="fft")
                    for out_ps, combos in (
                        (zre, [("re", "gre"), ("im", "gimn")]),
                        (zim, [("re", "gim"), ("im", "gre")]),
                    ):
                        i = 0
                        for nm, rt in combos:
                            for kt in range(2):
                                nc.tensor.matmul(
                                    out_ps,
                                    yts[(nm, kt)][:, mt * 128:(mt + 1) * 128],
                                    cs[rt][:, kt, :],
                                    start=(i == 0), stop=(i == 3))
                                i += 1
                    t1 = wpool.tile([128, 256], F32, name="mag1", tag="mag1")
                    t2 = wpool.tile([128, 256], F32, name="mag2", tag="mag2")
                    nc.scalar.square(t1, zre)
                    nc.scalar.square(t2, zim)
                    nc.vector.tensor_add(t1, t1, t2)
                    nc.scalar.sqrt(xm_sb[:, mt, :], t1)

                # xm -> DRAM bounce (rows 1..256, cols 1..256)
                nc.sync.dma_start(
                    out=dramap(xmd[img % 2], 258 + 1,
                               [[258, 128], [128 * 258, 2], [1, 256]]),
                    in_=xm_sb)

            def fill_strip1(img):
                # strip1 fills: K=36 dup layout, one DMA per (dx, dup copy)
                for dx in range(3):
                    for sp in range(2):
                        nc.gpsimd.dma_start(
                            out=mk(strip1, sp * 18 + dx * 6, 1, 6,
                                   [[256, 64], [1, 256]]),
                            in_=dramap(xmd[img % 2], dx,
                                       [[258, 6], [1032, 64], [1, 256]]))

            def conv_stage(img, b_loc):
                # ---- conv1: 32 chunks of 8 output rows
                for ch in range(32):
                    ps = convps.tile([128, 512], F32, name="c1ps", tag="conv")
                    nc.tensor.matmul(
                        ps, cs["w1l"],
                        mk(strip1, 0, 1, 36, [[256, 2], [1, 256]], 2 * ch * 256),
                        start=True, stop=True)
                    # x-pool off PSUM: copy even cols, max in odd cols (each
                    # DVE op reads at most one PSUM operand)
                    s1 = wpool.tile([128, 256], F32, name="s1", tag="s1", bufs=3)
                    nc.vector.tensor_copy(
                        s1, mk(ps, 0, 1, 128, [[256, 2], [2, 128]], 0))
                    nc.vector.tensor_max(
                        s1, s1, mk(ps, 0, 1, 128, [[256, 2], [2, 128]], 1))
                    nc.scalar.activation(
                        mk(xpooled1, 0, 1, 128, [[129, 2], [1, 128]],
                           2 * ch * 129),
                        s1, ActFn.Relu, bias=cs["cb1"][:, 0:1])
                # y-pool -> h1X [64 = (parity b, co), 64 s, 129]
                nc.sync.dma_start(
                    out=xpB1, in_=mk(xpooled1, 64, 1, 64, [[1, 8256]]))
                nc.vector.tensor_max(
                    h1X, mk(xpooled1, 0, 1, 64, [[1, 8256]]), xpB1)

                # ---- conv2 staging: 4 contiguous SBUF fills
                # (j, dest-row-offset, n-rows, src-parity, src-row-offset)
                for j, dro, nr, g, sro in ((0, 1, 63, 1, 0), (1, 0, 64, 0, 0),
                                           (2, 0, 64, 1, 0), (3, 0, 63, 0, 1)):
                    nc.gpsimd.dma_start(
                        out=mk(strip2, 32 * j, 1, 32, [[1, nr * 129]],
                               1 + dro * 129),
                        in_=mk(h1X, 32 * g, 1, 32, [[1, nr * 129]], sro * 129))
                if img + 1 < b_loc:
                    fill_strip1(img + 1)
                for ch in range(16):
                    ps = convps.tile([128, 512], F32, name="c2ps", tag="conv")
                    i = 0
                    for dx in range(3):
                        for sp in range(2):
                            nc.tensor.matmul(
                                ps, cs["w2l"][:, dx, sp, :],
                                mk(strip2, 0, 1, 128, [[129, 4], [1, 128]],
                                   4 * ch * 129 + dx),
                                start=(i == 0), stop=(i == 5))
                            i += 1
                    s2 = wpool.tile([128, 256], F32, name="s2", tag="s2", bufs=3)
                    nc.vector.tensor_copy(
                        s2, mk(ps, 0, 1, 128, [[128, 4], [2, 64]], 0))
                    nc.vector.tensor_max(
                        s2, s2, mk(ps, 0, 1, 128, [[128, 4], [2, 64]], 1))
                    nc.scalar.activation(
                        mk(xpooled2, 0, 1, 128, [[65, 4], [1, 64]],
                           4 * ch * 65),
                        s2, ActFn.Relu, bias=cs["cb2"][:, 0:1])
                nc.sync.dma_start(
                    out=xpB2, in_=mk(xpooled2, 64, 1, 64, [[1, 4160]]))
                nc.vector.tensor_max(
                    h2buf, mk(xpooled2, 0, 1, 64, [[1, 4160]]), xpB2)

                # ---- conv3 staging: dup-fill (one DMA per copy)
                for sp in range(2):
                    nc.gpsimd.dma_start(
                        out=mk(strip3, 64 * sp, 1, 64, [[1, 4160]], 66),
                        in_=h2buf)

                qacc = wpool.tile([128, 8], F32, name="qacc", tag="qacc")
                for ch in range(8):
                    ps = convps.tile([128, 512], F32, name="c3ps", tag="conv")
                    i = 0
                    for dy in range(3):
                        for dx in range(3):
                            nc.tensor.matmul(
                                ps, cs["w3t"][:, dy * 3 + dx, :],
                                mk(strip3, 0, 1, 128, [[65, 8], [1, 64]],
                                   (8 * ch + dy) * 65 + dx),
                                start=(i == 0), stop=(i == 8))
                            i += 1
                    scr = wpool.tile([128, 512], F16, name="scr3", tag="scr3")
                    nc.scalar.activation(scr, ps, ActFn.Relu,
                                         bias=cs["cb3"][:, 0:1],
                                         accum_out=qacc[:, ch:ch + 1])
                nc.vector.reduce_sum(qT[:, img:img + 1], qacc, axis=AxX)

            for i in range(b_loc + 1):
                if i < b_loc:
                    fft_stage(i)
                if i == 0:
                    fill_strip1(0)
                if i > 0:
                    conv_stage(i - 1, b_loc)

            # ---------------- retrieval (fp32) ----------------
            bl = b_loc
            simps = miscps.tile([bl, 400], F32, name="simps", tag="misc")
            nc.tensor.matmul(simps, qT, knT, start=True, stop=True)
            gram = miscps.tile([bl, bl], F32, name="gram", tag="misc")
            nc.tensor.matmul(gram, qT, qT, start=True, stop=True)
            gd = rpool.tile([bl, bl], F32, name="gd")
            nc.vector.tensor_mul(gd, gram, cs["ident"][:bl, :bl])
            q2 = rpool.tile([bl, 1], F32, name="q2")
            nc.vector.reduce_sum(q2, gd, axis=AxX)
            qn = rpool.tile([bl, 1], F32, name="qn")
            nc.scalar.sqrt(qn, q2)
            nc.vector.tensor_scalar_max(qn, qn, 1e-12)
            rq = rpool.tile([bl, 1], F32, name="rq")
            nc.vector.reciprocal(rq, qn)
            sim = rpool.tile([bl, 400], F32, name="sim")
            nc.vector.tensor_scalar_mul(sim, simps, rq[:, 0:1])

            cur = rpool.tile([bl, 400], F32, name="cur")
            nc.vector.tensor_copy(cur, sim)
            m1 = rpool.tile([bl, 1], F32, name="m1")
            nc.vector.reduce_max(m1, sim, axis=AxX)
            msk = rpool.tile([bl, 400], F32, name="msk")
            mk_ = m1
            for it in range(4):
                nc.vector.tensor_scalar(msk, cur, mk_[:, 0:1], None,
                                        op0=AluOp.is_ge)
                nc.vector.scalar_tensor_tensor(cur, msk, -1e30, cur,
                                               op0=AluOp.mult, op1=AluOp.add)
                nm_ = rpool.tile([bl, 1], F32, name=f"mk{it}")
                nc.vector.reduce_max(nm_, cur, axis=AxX)
                mk_ = nm_
            m5 = mk_
            nc.vector.tensor_scalar(msk, sim, m5[:, 0:1], None, op0=AluOp.is_ge)
            m1n = rpool.tile([bl, 1], F32, name="m1n")
            nc.vector.tensor_scalar_mul(m1n, m1, -1.0)
            es = rpool.tile([bl, 400], F32, name="es")
            nc.scalar.activation(es, sim, ActFn.Exp, bias=m1n[:, 0:1])
            ew = rpool.tile([bl, 400], F32, name="ew")
            nc.vector.tensor_mul(ew, es, msk)
            zs = rpool.tile([bl, 1], F32, name="zs")
            nc.vector.reduce_sum(zs, ew, axis=AxX)
            rz = rpool.tile([bl, 1], F32, name="rz")
            nc.vector.reciprocal(rz, zs)
            nc.vector.tensor_scalar_mul(ew, ew, rz[:, 0:1])

            eT = rpool.tile([128, 4, bl], F32, name="eT")
            for c in range(4):
                pc = 128 if c < 3 else 16
                tp = miscps.tile([128, bl], F32, name="tp_e", tag="misc")
                nc.tensor.transpose(tp[:pc, :], ew[:, c * 128:c * 128 + pc],
                                    cs["ident"][:bl, :bl])
                nc.scalar.copy(eT[:pc, c, :], tp[:pc, :])

            memps = miscps.tile([128, bl], F32, name="memps", tag="misc")
            for c in range(4):
                pc = 128 if c < 3 else 16
                nc.tensor.matmul(memps, cs["vals"][:pc, c, :], eT[:pc, c, :],
                                 start=(c == 0), stop=(c == 3))
            memT = rpool.tile([128, bl], F32, name="memT")
            nc.scalar.copy(memT, memps)

            h1T = rpool.tile([128, 2, bl], F32, name="h1T")
            for mt in range(2):
                ps = miscps.tile([128, bl], F32, name="d1ps", tag="misc")
                nc.tensor.matmul(ps, cs["w1d"][:, mt * 128:(mt + 1) * 128],
                                 memT, start=True, stop=True)
                nc.scalar.activation(h1T[:, mt, :], ps, ActFn.Relu,
                                     bias=cs["b1d"][:, mt:mt + 1])
            h2T = rpool.tile([128, 4, bl], F32, name="h2T")
            for mt in range(4):
                ps = miscps.tile([128, bl], F32, name="d2ps", tag="misc")
                for kt in range(2):
                    nc.tensor.matmul(ps, cs["w2d"][:, kt, mt, :], h1T[:, kt, :],
                                     start=(kt == 0), stop=(kt == 1))
                nc.scalar.activation(h2T[:, mt, :], ps, ActFn.Relu,
                                     bias=cs["b2d"][:, mt:mt + 1])
            ops = miscps.tile([bl, 16], F32, name="outps", tag="misc")
            for c in range(4):
                nc.tensor.matmul(ops, h2T[:, c, :], cs["w3d"][:, c, :],
                                 start=(c == 0), stop=False)
            nc.tensor.matmul(ops, cs["ones1"][:, :bl], cs["b3row"],
                             start=False, stop=True)
            out_sb = rpool.tile([bl, 16], F32, name="out_sb")
            nc.scalar.copy(out_sb, ops)
            nc.sync.dma_start(out=out_d.ap(), in_=out_sb)

    nc.compile()
    return nc


# ---------------------------------------------------------------------------
# host entry
# ---------------------------------------------------------------------------
_NC_CACHE = {}


def _get_nc(b_loc):
    if b_loc not in _NC_CACHE:
        _NC_CACHE[b_loc] = build_nc(b_loc)
    return _NC_CACHE[b_loc]


def _pack_x(x_shard):
    b = x_shard.shape[0]
    xr = np.ascontiguousarray(
        x_shard.reshape(b, 2, 128, 256).transpose(0, 2, 1, 3)).astype(np.float32)
    return xr.astype(np.float16)


def kernel(**inputs):
    x = np.asarray(inputs["x"], np.float32)
    # jnp.fft.fftshift also shifts the batch axis: output b uses x[(b+64)%128]
    xp = np.roll(x, -64, axis=0)
    consts = _host_consts(inputs)

    b_loc = B // N_CORES
    nc = _get_nc(b_loc)

    in_maps = []
    for c in range(N_CORES):
        m = dict(consts)
        m["x_in"] = _pack_x(xp[c * b_loc:(c + 1) * b_loc])
        in_maps.append(m)

    kwargs = {}
    if os.environ.get("K_TRACE"):
        kwargs["trace"] = True
        try:  # dev-only: register the axon NTFF hook missing from the image
            import types
            import antenv
            if not hasattr(antenv, "axon_hooks"):
                import trn_agent_boot.trn_boot as _tb
                _hook = _tb._ntff_profile_via_ctypes("/opt/axon/libaxon_pjrt.so")
                _m = types.ModuleType("antenv.axon_hooks")
                _hh = [_hook]
                _m.set_axon_ntff_profile_hook = lambda h: _hh.__setitem__(0, h)
                _m.get_axon_ntff_profile_hook = lambda: _hh[0]
                sys.modules["antenv.axon_hooks"] = _m
                antenv.axon_hooks = _m
        except Exception as e:
            print("trace hook shim failed:", e)
    res = run_bass_kernel_spmd(nc, in_maps, core_ids=list(range(N_CORES)),
                               **kwargs)
    global LAST_RESULTS
    LAST_RESULTS = res
    out = np.concatenate([r["out"] for r in res.results], axis=0)
    return out.reshape(B, 1, 4, 4).astype(np.float32)


LAST_RESULTS = None


if __name__ == "__main__":
    build_nc(int(os.environ.get("K_BLOC", "16")))
    print("built ok")
